# revision 35
# baseline (speedup 1.0000x reference)
"""AdaZero encoder layer on 8 Trainium2 NeuronCores.

Sharding: zero-collective hybrid. Core c handles batch b = c // 2 and
query-row half h = c % 2 (512 of the 1024 sequence rows). Each core
computes the full K/V for its batch (duplicated across the 2 cores of a
batch, ~14% extra FLOPs) and attention + FFN for its own 512 query rows,
so no inter-core communication is needed. The graph is SPMD-identical
across cores: per-core differences are pushed into the data by rolling
the sequence axis on the host and passing rolled RoPE tables.

Compute dtype: bf16 matmuls with fp32 PSUM accumulation; layernorm stats
and the residual stream stay fp32. The adaLN gates scale the sublayer
outputs by ~0.02, so bf16 error on the sublayer path is ~1e-4 relative
on the final output.
"""

import os
import sys
import types

import numpy as np
import ml_dtypes

D_MODEL = 1024
HEADS = 16
HEAD_DIM = 64
D_FF = 4096
GAMMA_SCALE = 1.0
LN_EPS = 1e-5
ROPE_BASE = 10000.0
B = 4
S = 1024
SH = 512  # query rows per core
N_CORES = 8

_BF16 = ml_dtypes.bfloat16

_graph_cache = {}


def _install_ntff_shim():
    """run_bass_kernel_spmd(trace=True) under axon needs antenv.axon_hooks;
    this image's antenv lacks it, but the ctypes impl lives in trn_agent_boot."""
    if "antenv.axon_hooks" in sys.modules:
        return
    import antenv
    mod = types.ModuleType("antenv.axon_hooks")
    store = {"h": None}
    mod.set_axon_ntff_profile_hook = lambda h: store.__setitem__("h", h)
    mod.get_axon_ntff_profile_hook = lambda: store["h"]
    sys.modules["antenv.axon_hooks"] = mod
    antenv.axon_hooks = mod
    try:
        from trn_agent_boot.trn_boot import _ntff_profile_via_ctypes
        hook = _ntff_profile_via_ctypes("/opt/axon/libaxon_pjrt.so")
        if hook is not None:
            mod.set_axon_ntff_profile_hook(hook)
    except Exception:
        pass


def _build_graph(flags):
    """Build the SPMD per-core Bass graph. `flags` = (mask_trivial, bq_nz,
    bk_nz, bv_nz, bo_nz, b1_nz, b2_nz): whether the mask is all-True and
    whether the (normally all-zero) bias paths are emitted."""
    import concourse.bass as bass
    import concourse.mybir as mybir
    import concourse.tile as tile
    from concourse import bacc
    from concourse.masks import make_identity
    from contextlib import ExitStack

    mtriv, bq_nz, bk_nz, bv_nz, bo_nz, b1_nz, b2_nz = flags
    fp32 = mybir.dt.float32
    bf16 = mybir.dt.bfloat16
    AF = mybir.ActivationFunctionType
    OP = mybir.AluOpType

    nc = bacc.Bacc(None, target_bir_lowering=False)

    # ---- DRAM parameters (per-core shards; all cores share shapes) ----
    x_d = nc.dram_tensor("x", [S, D_MODEL], fp32, kind="ExternalInput")
    wq_d = nc.dram_tensor("wq", [8, 128, 1024], bf16, kind="ExternalInput")  # lhsT-tiled
    wk_d = nc.dram_tensor("wk", [8, 128, 1024], bf16, kind="ExternalInput")  # lhsT-tiled
    wv_d = nc.dram_tensor("wv", [8, 128, 1024], bf16, kind="ExternalInput")  # natural rows
    wo_d = nc.dram_tensor("wo", [8, 128, 1024], bf16, kind="ExternalInput")  # natural rows
    w1_d = nc.dram_tensor("w1", [32, 128, 1024], bf16, kind="ExternalInput")  # lhsT-tiled
    w2_d = nc.dram_tensor("w2", [32, 128, 1024], bf16, kind="ExternalInput")  # natural rows
    cos_d = nc.dram_tensor("cos2", [128, S], bf16, kind="ExternalInput")
    sin_d = nc.dram_tensor("sin2", [128, S], bf16, kind="ExternalInput")
    mod_d = nc.dram_tensor("mod", [6, D_MODEL], bf16, kind="ExternalInput")
    maskb_d = nc.dram_tensor("maskb", [128, 8], fp32, kind="ExternalInput")
    out_d = nc.dram_tensor("out", [SH, D_MODEL], fp32, kind="ExternalOutput")
    bias_d = {}
    if bq_nz:
        bias_d["bq"] = nc.dram_tensor("bq", [128, 8], fp32, kind="ExternalInput")
    if bk_nz:
        bias_d["bk"] = nc.dram_tensor("bk", [128, 8], fp32, kind="ExternalInput")
    if bv_nz:
        bias_d["bv"] = nc.dram_tensor("bv", [D_MODEL], fp32, kind="ExternalInput")
    if bo_nz:
        bias_d["bo"] = nc.dram_tensor("bo", [D_MODEL], fp32, kind="ExternalInput")
    if b1_nz:
        bias_d["b1"] = nc.dram_tensor("b1", [128, 32], fp32, kind="ExternalInput")
    if b2_nz:
        bias_d["b2"] = nc.dram_tensor("b2", [D_MODEL], fp32, kind="ExternalInput")

    with ExitStack() as ctx:
        tc = ctx.enter_context(tile.TileContext(nc))

        const = ctx.enter_context(tc.tile_pool(name="const", bufs=1))
        ident = const.tile([128, 128], bf16)
        make_identity(nc, ident[:])
        cos2 = const.tile([128, S], bf16)
        nc.gpsimd.dma_start(out=cos2[:], in_=cos_d[:])
        sin2 = const.tile([128, S], bf16)
        nc.gpsimd.dma_start(out=sin2[:], in_=sin_d[:])
        maskb = const.tile([128, 8], fp32)
        nc.gpsimd.dma_start(out=maskb[:], in_=maskb_d[:])
        eps_t = const.tile([128, 1], fp32)
        nc.vector.memset(eps_t[:], LN_EPS)
        ones_k = const.tile([128, 64], bf16)
        nc.vector.memset(ones_k[:], 1.0)
        # adaLN modulation vectors, broadcast across partitions
        mods = []
        for i in range(6):
            m = const.tile([128, D_MODEL], bf16, tag=f"mod{i}")
            nc.gpsimd.dma_start(out=m[:], in_=bass.AP(tensor=mod_d, offset=i * D_MODEL,
                                                      ap=[[0, 128], [1, D_MODEL]]))
            mods.append(m)
        m0b, b0b, a0b, m1b, b1mb, a1b = mods
        bias_sb = {}
        for nm in ("bq", "bk", "b1"):
            if nm in bias_d:
                t = const.tile(list(bias_d[nm].shape), fp32, tag=f"bias_{nm}")
                nc.sync.dma_start(out=t[:], in_=bias_d[nm][:])
                bias_sb[nm] = t
        for nm in ("bv", "bo", "b2"):
            if nm in bias_d:
                t = const.tile([128, D_MODEL], fp32, tag=f"bias_{nm}")
                nc.sync.dma_start(out=t[:], in_=bass.AP(tensor=bias_d[nm], offset=0,
                                                        ap=[[0, 128], [1, D_MODEL]]))
                bias_sb[nm] = t

        x_q = ctx.enter_context(tc.tile_pool(name="xq", bufs=1)).tile(
            [128, 4, D_MODEL], fp32)
        x1 = ctx.enter_context(tc.tile_pool(name="x1", bufs=1)).tile(
            [128, 4, D_MODEL], fp32)
        O_sb = ctx.enter_context(tc.tile_pool(name="attnO", bufs=1)).tile(
            [128, 8, SH], bf16)  # O^T concat [d, q]

        def layernorm_mod(x_t, pool, mbt, bbt, tagsfx):
            """LN over free axis + adaLN modulation; returns bf16 [128, D]."""
            stats = pool.tile([128, 2, 6], fp32, tag="stats" + tagsfx)
            nc.vector.bn_stats(out=stats[:, 0, :], in_=x_t[:, 0:512])
            nc.vector.bn_stats(out=stats[:, 1, :], in_=x_t[:, 512:1024])
            mv = pool.tile([128, 2], fp32, tag="mv" + tagsfx)
            nc.vector.bn_aggr(out=mv[:], in_=stats[:])
            std = pool.tile([128, 1], fp32, tag="std" + tagsfx)
            nc.scalar.activation(std[:], mv[:, 1:2], AF.Sqrt, bias=eps_t[:])
            rstd = pool.tile([128, 1], fp32, tag="rstd" + tagsfx)
            nc.vector.reciprocal(rstd[:], std[:])
            nrm = pool.tile([128, D_MODEL], bf16, tag="nrm" + tagsfx)
            nc.vector.tensor_scalar(out=nrm[:], in0=x_t, scalar1=mv[:, 0:1],
                                    scalar2=rstd[:], op0=OP.subtract, op1=OP.mult)
            t1 = pool.tile([128, D_MODEL], bf16, tag="t1" + tagsfx)
            nc.vector.tensor_mul(t1[:], nrm[:], mbt[:])
            nm_ = pool.tile([128, D_MODEL], bf16, tag="nm" + tagsfx)
            nc.vector.tensor_add(nm_[:], t1[:], bbt[:])
            return nm_

        def rope_apply(dst, src, n, pool):
            # dst, src: [128, n] bf16; rotate-half RoPE with sign-folded tables.
            # The rotate-half partition swap must go through DMA (DVE lanes
            # are partition-locked); spread the 4 slab copies over two queues.
            swp = pool.tile([128, n], bf16, tag="ropeswp")
            for eng, lo, sl in ((nc.gpsimd, 0, 32), (nc.scalar, 32, 0),
                                (nc.gpsimd, 64, 96), (nc.scalar, 96, 64)):
                eng.dma_start(out=swp[lo:lo + 32, :], in_=src[sl:sl + 32, :])
            tcos = pool.tile([128, n], bf16, tag="ropecos")
            nc.vector.tensor_mul(tcos[:], src, cos2[:, 0:n])
            tsin = pool.tile([128, n], bf16, tag="ropesin")
            nc.vector.tensor_mul(tsin[:], swp[:], sin2[:, 0:n])
            nc.vector.tensor_add(dst, tcos[:], tsin[:])

        with tc.tile_pool(name="kqv", bufs=1) as kqvp:
            Qt = kqvp.tile([128, 8, SH], bf16)       # Q~^T: [do, q]
            Kt = kqvp.tile([128, 8, S], bf16)        # K~^T: [do, k]
            Vn = kqvp.tile([128, 8, HEADS, HEAD_DIM], bf16)  # V natural

            with tc.tile_pool(name="n1t", bufs=1) as n1tp:
                n1T = n1tp.tile([128, 8, 1024], bf16)   # n1^T: [d, s]

                # ---------- Phase A: LN1 + modulation + transpose ----------
                with tc.tile_pool(name="ln1", bufs=3) as ln1p, \
                     tc.tile_pool(name="ln1ps", bufs=2, space="PSUM") as lnps:
                    for st in range(8):
                        if st < 4:
                            x_t = x_q[:, st, :]
                        else:
                            xkv = ln1p.tile([128, D_MODEL], fp32, tag="xkv")
                            x_t = xkv[:]
                        nc.sync.dma_start(out=x_t[:, 0:512],
                                          in_=x_d[st * 128:(st + 1) * 128, 0:512])
                        nc.scalar.dma_start(out=x_t[:, 512:1024],
                                            in_=x_d[st * 128:(st + 1) * 128, 512:1024])
                        n1m = layernorm_mod(x_t, ln1p, m0b, b0b, "1")
                        for dt in range(8):
                            tps = lnps.tile([128, 128], bf16, tag="tps")
                            nc.tensor.transpose(tps[:], n1m[:, dt * 128:(dt + 1) * 128],
                                                ident[:])
                            nc.scalar.copy(out=n1T[:, dt, st * 128:(st + 1) * 128],
                                           in_=tps[:])

                # ---------- Phase B: QKV projections (+RoPE) ----------
                with tc.tile_pool(name="wstream", bufs=3) as wsp, \
                     tc.tile_pool(name="wv", bufs=1) as wvp, \
                     tc.tile_pool(name="qkvtmp", bufs=3) as qtp, \
                     tc.tile_pool(name="qkvps", bufs=3, space="PSUM") as qps:
                    # Q^T[do, q], q = first 512 rolled rows
                    for m in range(8):
                        wqt = wsp.tile([128, 1024], bf16, tag="wt")
                        (nc.sync if m % 2 else nc.gpsimd).dma_start(
                            out=wqt[:], in_=wq_d[m])
                        ps = qps.tile([128, SH], fp32, tag="projps")
                        for k in range(8):
                            nc.tensor.matmul(ps[:], wqt[:, k * 128:(k + 1) * 128],
                                             n1T[:, k, 0:SH],
                                             start=(k == 0), stop=(k == 7))
                        qtmp = qtp.tile([128, SH], bf16, tag="qtmp")
                        if bq_nz:
                            nc.scalar.activation(qtmp[:], ps[:], AF.Copy,
                                                 bias=bias_sb["bq"][:, m:m + 1])
                        else:
                            nc.scalar.copy(out=qtmp[:], in_=ps[:])
                        rope_apply(Qt[:, m, :], qtmp[:], SH, qtp)

                    # K^T[do, k] over all 1024 rolled rows
                    for m in range(8):
                        wkt = wsp.tile([128, 1024], bf16, tag="wt")
                        (nc.sync if m % 2 else nc.gpsimd).dma_start(
                            out=wkt[:], in_=wk_d[m])
                        ktmp = qtp.tile([128, S], bf16, tag="ktmp")
                        for nh in range(2):
                            ps = qps.tile([128, SH], fp32, tag="projps")
                            for k in range(8):
                                nc.tensor.matmul(ps[:], wkt[:, k * 128:(k + 1) * 128],
                                                 n1T[:, k, nh * SH:(nh + 1) * SH],
                                                 start=(k == 0), stop=(k == 7))
                            if bk_nz:
                                nc.scalar.activation(ktmp[:, nh * SH:(nh + 1) * SH],
                                                     ps[:], AF.Copy,
                                                     bias=bias_sb["bk"][:, m:m + 1])
                            else:
                                nc.scalar.copy(out=ktmp[:, nh * SH:(nh + 1) * SH],
                                               in_=ps[:])
                        rope_apply(Kt[:, m, :], ktmp[:], S, qtp)

                    # V natural [s, dv] (+ ones column per head)
                    wv_sb = wvp.tile([128, 8, 1024], bf16)
                    for k in range(8):
                        nc.sync.dma_start(out=wv_sb[:, k, :], in_=wv_d[k])
                    for st in range(8):
                        for nh in range(2):
                            ps = qps.tile([128, SH], fp32, tag="projps")
                            for k in range(8):
                                nc.tensor.matmul(ps[:],
                                                 n1T[:, k, st * 128:(st + 1) * 128],
                                                 wv_sb[:, k, nh * SH:(nh + 1) * SH],
                                                 start=(k == 0), stop=(k == 7))
                            src = ps[:]
                            if bv_nz:
                                vtmp = qtp.tile([128, SH], fp32, tag="vtmp")
                                nc.vector.tensor_add(
                                    vtmp[:], ps[:],
                                    bias_sb["bv"][:, nh * SH:(nh + 1) * SH])
                                src = vtmp[:]
                            nc.vector.tensor_copy(
                                out=Vn[:, st, nh * 8:(nh + 1) * 8, :],
                                in_=src.rearrange("p (h d) -> p h d", d=HEAD_DIM))

            # ---------- Phase C: attention ----------
            # Heads run in even/odd pairs. Per pair: scores for both heads
            # (PE row groups 0:64 / 64:128 work concurrently), exp on 2-bank
            # PSUM tiles, then attnV + denominator MMs packed into single
            # [128,512] PSUM banks (even head rows 0:64, odd head rows 64:128
            # via tile_position) so normalization is ONE wide reciprocal and
            # ONE wide multiply writing O_sb directly.
            with tc.tile_pool(name="pt", bufs=2) as ptp, \
                 tc.tile_pool(name="dn", bufs=2) as dnp, \
                 tc.tile_pool(name="spsp", bufs=2, space="PSUM") as spsp, \
                 tc.tile_pool(name="ovdn", bufs=4, space="PSUM") as ovdnp:
                for pr in range(HEADS // 2):
                    mt = pr
                    PTs = [ptp.tile([128, 8, SH], bf16, tag=f"PT{par}",
                                    name=f"PT_{pr}_{par}") for par in range(2)]
                    for kb2 in range(4):
                        sps2 = [spsp.tile([128, 2, SH], fp32, tag="sps",
                                          name=f"sps_{pr}_{kb2}_{par}")
                                for par in range(2)]
                        for sub in range(2):
                            kb = 2 * kb2 + sub
                            for par in range(2):
                                po = par * 64
                                nc.tensor.matmul(
                                    sps2[par][:, sub, :],
                                    Kt[po:po + 64, mt, kb * 128:(kb + 1) * 128],
                                    Qt[po:po + 64, mt, :])
                        for par in range(2):
                            if mtriv:
                                nc.scalar.activation(
                                    PTs[par][:, 2 * kb2:2 * kb2 + 2, :],
                                    sps2[par][:], AF.Exp,
                                    scale=float(1.0 / np.sqrt(HEAD_DIM)))
                            else:
                                for sub in range(2):
                                    kb = 2 * kb2 + sub
                                    nc.scalar.activation(
                                        PTs[par][:, kb, :], sps2[par][:, sub, :],
                                        AF.Exp, bias=maskb[:, kb:kb + 1],
                                        scale=float(1.0 / np.sqrt(HEAD_DIM)))
                    ov2 = ovdnp.tile([128, SH], fp32, tag="ovdn",
                                     name=f"ov_{pr}")
                    dn2 = ovdnp.tile([128, SH], fp32, tag="ovdn",
                                     name=f"dn_{pr}")
                    for kb in range(8):
                        st = (kb == 0)
                        sp = (kb == 7)
                        nc.tensor.matmul(ov2[0:64, :], Vn[:, kb, 2 * pr, :],
                                         PTs[0][:, kb, :], start=st, stop=sp,
                                         skip_group_check=True)
                        nc.tensor.matmul(ov2[64:128, :], Vn[:, kb, 2 * pr + 1, :],
                                         PTs[1][:, kb, :], start=st, stop=sp,
                                         tile_position=(0, 64),
                                         skip_group_check=True)
                        nc.tensor.matmul(dn2[0:64, :], ones_k[:],
                                         PTs[0][:, kb, :], start=st, stop=sp,
                                         skip_group_check=True)
                        nc.tensor.matmul(dn2[64:128, :], ones_k[:],
                                         PTs[1][:, kb, :], start=st, stop=sp,
                                         tile_position=(0, 64),
                                         skip_group_check=True)
                    rrec = dnp.tile([128, SH], fp32, tag="rrec",
                                    name=f"rrec_{pr}")
                    nc.vector.reciprocal(rrec[:], dn2[:])
                    nc.vector.tensor_mul(O_sb[:, mt, :], ov2[:], rrec[:])

        # ---------- Phase D: output projection + residual + LN2,
        # interleaved per qb-block so LN2's DVE work and transposes overlap
        # the next qb's out-projection matmuls ----------
        with tc.tile_pool(name="ffn", bufs=1) as ffnp:
            n2T = ffnp.tile([128, 8, SH], bf16)
            hT = ffnp.tile([128, 32, SH], bf16)

            with tc.tile_pool(name="wo", bufs=1) as wop, \
                 tc.tile_pool(name="optmp", bufs=3) as opt, \
                 tc.tile_pool(name="ln2", bufs=2) as ln2p, \
                 tc.tile_pool(name="ops", bufs=3, space="PSUM") as opsp, \
                 tc.tile_pool(name="ln2ps", bufs=2, space="PSUM") as ln2ps:
                wo_sb = wop.tile([128, 8, 1024], bf16)
                for k in range(8):
                    (nc.sync if k % 2 else nc.scalar).dma_start(
                        out=wo_sb[:, k, :], in_=wo_d[k])
                for qb in range(4):
                    for nh in range(2):
                        ps = opsp.tile([128, SH], fp32, tag="ops")
                        for k in range(8):
                            nc.tensor.matmul(ps[:],
                                             O_sb[:, k, qb * 128:(qb + 1) * 128],
                                             wo_sb[:, k, nh * SH:(nh + 1) * SH],
                                             start=(k == 0), stop=(k == 7))
                        sl = slice(nh * SH, (nh + 1) * SH)
                        t = opt.tile([128, SH], fp32, tag="opt")
                        if bo_nz:
                            t2 = opt.tile([128, SH], fp32, tag="opt2")
                            nc.vector.tensor_add(t2[:], ps[:], bias_sb["bo"][:, sl])
                            nc.vector.tensor_mul(t[:], t2[:], a0b[:, sl])
                        else:
                            nc.vector.tensor_mul(t[:], ps[:], a0b[:, sl])
                        nc.vector.tensor_add(x1[:, qb, sl], t[:], x_q[:, qb, sl])
                    n2m = layernorm_mod(x1[:, qb, :], ln2p, m1b, b1mb, "2")
                    for dt in range(8):
                        tps = ln2ps.tile([128, 128], bf16, tag="tps2")
                        nc.tensor.transpose(tps[:], n2m[:, dt * 128:(dt + 1) * 128],
                                            ident[:])
                        nc.scalar.copy(out=n2T[:, dt, qb * 128:(qb + 1) * 128],
                                       in_=tps[:])

            # FFN1: hT[dff, q] = gelu(w1^T @ n2^T)
            with tc.tile_pool(name="w1s", bufs=3) as w1p, \
                 tc.tile_pool(name="f1ps", bufs=2, space="PSUM") as f1ps:
                for j in range(32):
                    w1t = w1p.tile([128, 1024], bf16, tag="w1t")
                    (nc.sync, nc.gpsimd, nc.scalar)[j % 3].dma_start(
                        out=w1t[:], in_=w1_d[j])
                    ps = f1ps.tile([128, SH], fp32, tag="f1")
                    for k in range(8):
                        nc.tensor.matmul(ps[:], w1t[:, k * 128:(k + 1) * 128],
                                         n2T[:, k, :], start=(k == 0), stop=(k == 7))
                    if b1_nz:
                        nc.scalar.activation(hT[:, j, :], ps[:], AF.Gelu,
                                             bias=bias_sb["b1"][:, j:j + 1])
                    else:
                        nc.scalar.activation(hT[:, j, :], ps[:], AF.Gelu)

            # FFN2: y[q, do] accumulated over j, in two qb-groups so the
            # first group's epilogue + output DMA overlap the second group's
            # matmuls (w2 is streamed twice; +8MB HBM, hidden).
            with tc.tile_pool(name="w2s", bufs=3) as w2p, \
                 tc.tile_pool(name="f2ps", bufs=1, space="PSUM") as f2ps, \
                 tc.tile_pool(name="otmp", bufs=2) as otp:
                for grp in range(2):
                    qbs = (0, 1) if grp == 0 else (2, 3)
                    psl = {(qb, nh): f2ps.tile([128, SH], fp32,
                                               tag=f"f2_{qb}_{nh}",
                                               name=f"f2_{qb}_{nh}")
                           for qb in qbs for nh in range(2)}
                    for j in range(32):
                        w2t = w2p.tile([128, 1024], bf16, tag="w2t")
                        (nc.sync, nc.gpsimd, nc.scalar)[(j + grp) % 3].dma_start(
                            out=w2t[:], in_=w2_d[j])
                        for qb in qbs:
                            for nh in range(2):
                                nc.tensor.matmul(psl[qb, nh][:],
                                                 hT[:, j, qb * 128:(qb + 1) * 128],
                                                 w2t[:, nh * SH:(nh + 1) * SH],
                                                 start=(j == 0), stop=(j == 31))
                    for qb in qbs:
                        for nh in range(2):
                            sl = slice(nh * SH, (nh + 1) * SH)
                            ps = psl[qb, nh]
                            t = otp.tile([128, SH], fp32, tag="ot")
                            if b2_nz:
                                t2 = otp.tile([128, SH], fp32, tag="ot2")
                                nc.vector.tensor_add(t2[:], ps[:],
                                                     bias_sb["b2"][:, sl])
                                nc.vector.tensor_mul(t[:], t2[:], a1b[:, sl])
                            else:
                                nc.vector.tensor_mul(t[:], ps[:], a1b[:, sl])
                            yo = otp.tile([128, SH], fp32, tag="yo")
                            nc.vector.tensor_add(yo[:], t[:], x1[:, qb, sl])
                            eng = (nc.sync, nc.gpsimd, nc.scalar)[(qb * 2 + nh) % 3]
                            eng.dma_start(out=out_d[qb * 128:(qb + 1) * 128, sl],
                                          in_=yo[:])

    nc.compile()
    return nc


def _lhsT_tile(w, nblocks_in, nblocks_out):
    # w: [in, out] -> [nblocks_out, 128, nblocks_in*128] with
    # result[m][p, k*128+c] = w[k*128+p, m*128+c]
    kin = w.shape[0] // nblocks_in
    return np.ascontiguousarray(
        w.reshape(nblocks_in, kin, nblocks_out, w.shape[1] // nblocks_out)
        .transpose(2, 1, 0, 3)
        .reshape(nblocks_out, kin, -1))


def kernel(src_reps, src_mask, compact_style,
           ada0_w, ada0_b, ada1_w, ada1_b,
           wq, bq, wk, bk, wv, bv, wo, bo,
           w1, b1, w2, b2):
    trace = bool(os.environ.get("KERNEL_TRACE"))
    if trace:
        _install_ntff_shim()
    from concourse.bass_utils import run_bass_kernel_spmd

    src_reps = np.asarray(src_reps, np.float32)
    src_mask = np.asarray(src_mask)
    compact_style = np.asarray(compact_style, np.float32)

    # ---- host prep: adaLN styles ----
    def styles(ada_w, ada_b):
        cs = compact_style
        silu = cs * (1.0 / (1.0 + np.exp(-cs)))
        st = silu @ np.asarray(ada_w, np.float32) + np.asarray(ada_b, np.float32)
        g, be, al = st[:, :D_MODEL], st[:, D_MODEL:2 * D_MODEL], st[:, 2 * D_MODEL:]
        return (1.0 + np.tanh(g) * GAMMA_SCALE), be, al

    m0, be0, al0 = styles(ada0_w, ada0_b)
    m1, be1, al1 = styles(ada1_w, ada1_b)

    # ---- host prep: weights (cast + tile) ----
    wq_l = _lhsT_tile(np.asarray(wq), 8, 8).astype(_BF16)
    wk_l = _lhsT_tile(np.asarray(wk), 8, 8).astype(_BF16)
    wv_n = np.ascontiguousarray(np.asarray(wv).reshape(8, 128, 1024)).astype(_BF16)
    wo_n = np.ascontiguousarray(np.asarray(wo).reshape(8, 128, 1024)).astype(_BF16)
    w1_l = _lhsT_tile(np.asarray(w1), 8, 32).astype(_BF16)
    w2_n = np.ascontiguousarray(np.asarray(w2).reshape(32, 128, 1024)).astype(_BF16)

    flags = (bool(np.all(src_mask)),) + tuple(
        bool(np.any(np.asarray(b) != 0)) for b in (bq, bk, bv, bo, b1, b2))
    if flags not in _graph_cache:
        _graph_cache[flags] = _build_graph(flags)
    nc = _graph_cache[flags]

    # ---- host prep: RoPE tables (per roll offset) ----
    inv_freq = 1.0 / (ROPE_BASE **
                      (np.arange(0, HEAD_DIM, 2, dtype=np.float32) / HEAD_DIM))
    d_in_head = np.arange(64)
    fidx = np.where(d_in_head < 32, d_in_head, d_in_head - 32)
    sign = np.where(d_in_head < 32, -1.0, 1.0).astype(np.float32)

    def rope_tables(roll):
        pos = np.roll(np.arange(S, dtype=np.float32), -roll)
        ang = pos[None, :] * inv_freq[fidx][:, None]  # [64, S]
        c = np.cos(ang).astype(np.float32)
        s_ = (np.sin(ang) * sign[:, None]).astype(np.float32)
        return (np.ascontiguousarray(np.concatenate([c, c], 0)).astype(_BF16),
                np.ascontiguousarray(np.concatenate([s_, s_], 0)).astype(_BF16))

    tables = [rope_tables(0), rope_tables(SH)]

    in_maps = []
    for c in range(N_CORES):
        b, h = c // 2, c % 2
        x_c = np.ascontiguousarray(np.roll(src_reps[b], -h * SH, axis=0))
        mb = np.where(np.roll(src_mask[b], -h * SH), 0.0, -60.0).astype(np.float32)
        mod = np.stack([m0[b], be0[b], al0[b], m1[b], be1[b], al1[b]])
        im = {
            "x": x_c, "wq": wq_l, "wk": wk_l, "wv": wv_n, "wo": wo_n,
            "w1": w1_l, "w2": w2_n,
            "cos2": tables[h][0], "sin2": tables[h][1],
            "mod": np.ascontiguousarray(mod.astype(_BF16)),
            "maskb": np.ascontiguousarray(mb.reshape(8, 128).T),
        }
        if flags[1]:
            im["bq"] = np.ascontiguousarray(np.asarray(bq, np.float32).reshape(8, 128).T)
        if flags[2]:
            im["bk"] = np.ascontiguousarray(np.asarray(bk, np.float32).reshape(8, 128).T)
        if flags[3]:
            im["bv"] = np.asarray(bv, np.float32)
        if flags[4]:
            im["bo"] = np.asarray(bo, np.float32)
        if flags[5]:
            im["b1"] = np.ascontiguousarray(np.asarray(b1, np.float32).reshape(32, 128).T)
        if flags[6]:
            im["b2"] = np.asarray(b2, np.float32)
        in_maps.append(im)

    res = run_bass_kernel_spmd(nc, in_maps, core_ids=list(range(N_CORES)),
                               trace=trace)
    kernel.last_result = res

    out = np.empty((B, S, D_MODEL), np.float32)
    for c in range(N_CORES):
        b, h = c // 2, c % 2
        out[b, h * SH:(h + 1) * SH, :] = res.results[c]["out"]
    return out


# revision 43
# speedup vs baseline: 1.2276x; 1.2276x over previous
"""AdaZero encoder layer on 8 Trainium2 NeuronCores.

Sharding: zero-collective hybrid. Core c handles batch b = c // 2 and
query-row half h = c % 2 (512 of the 1024 sequence rows). Each core
computes the full K/V for its batch (duplicated across the 2 cores of a
batch, ~14% extra FLOPs) and attention + FFN for its own 512 query rows,
so no inter-core communication is needed. The graph is SPMD-identical
across cores: per-core differences are pushed into the data by rolling
the sequence axis on the host and passing rolled RoPE tables.

Compute dtype: bf16 matmuls with fp32 PSUM accumulation; layernorm stats
and the residual stream stay fp32. The adaLN gates scale the sublayer
outputs by ~0.02, so bf16 error on the sublayer path is ~1e-4 relative
on the final output.
"""

import os
import sys
import types

import numpy as np
import ml_dtypes

D_MODEL = 1024
HEADS = 16
HEAD_DIM = 64
D_FF = 4096
GAMMA_SCALE = 1.0
LN_EPS = 1e-5
ROPE_BASE = 10000.0
B = 4
S = 1024
SH = 512  # query rows per core
N_CORES = 8

_BF16 = ml_dtypes.bfloat16

_graph_cache = {}


def _install_ntff_shim():
    """run_bass_kernel_spmd(trace=True) under axon needs antenv.axon_hooks;
    this image's antenv lacks it, but the ctypes impl lives in trn_agent_boot."""
    if "antenv.axon_hooks" in sys.modules:
        return
    import antenv
    mod = types.ModuleType("antenv.axon_hooks")
    store = {"h": None}
    mod.set_axon_ntff_profile_hook = lambda h: store.__setitem__("h", h)
    mod.get_axon_ntff_profile_hook = lambda: store["h"]
    sys.modules["antenv.axon_hooks"] = mod
    antenv.axon_hooks = mod
    try:
        from trn_agent_boot.trn_boot import _ntff_profile_via_ctypes
        hook = _ntff_profile_via_ctypes("/opt/axon/libaxon_pjrt.so")
        if hook is not None:
            mod.set_axon_ntff_profile_hook(hook)
    except Exception:
        pass


def _build_graph(flags):
    """Build the SPMD per-core Bass graph. `flags` = (mask_trivial, bq_nz,
    bk_nz, bv_nz, bo_nz, b1_nz, b2_nz): whether the mask is all-True and
    whether the (normally all-zero) bias paths are emitted."""
    import concourse.bass as bass
    import concourse.mybir as mybir
    import concourse.tile as tile
    from concourse import bacc
    from concourse.masks import make_identity
    from contextlib import ExitStack

    mtriv, bq_nz, bk_nz, bv_nz, bo_nz, b1_nz, b2_nz = flags
    fp32 = mybir.dt.float32
    bf16 = mybir.dt.bfloat16
    AF = mybir.ActivationFunctionType
    OP = mybir.AluOpType

    nc = bacc.Bacc(None, target_bir_lowering=False)

    # ---- DRAM parameters (per-core shards; all cores share shapes) ----
    x_d = nc.dram_tensor("x", [S, D_MODEL], fp32, kind="ExternalInput")
    wq_d = nc.dram_tensor("wq", [8, 128, 1024], bf16, kind="ExternalInput")  # lhsT-tiled
    wk_d = nc.dram_tensor("wk", [8, 128, 1024], bf16, kind="ExternalInput")  # lhsT-tiled
    wv_d = nc.dram_tensor("wv", [8, 128, 1024], bf16, kind="ExternalInput")  # natural rows
    wo_d = nc.dram_tensor("wo", [8, 128, 1024], bf16, kind="ExternalInput")  # natural rows
    w1_d = nc.dram_tensor("w1", [32, 128, 1024], bf16, kind="ExternalInput")  # lhsT-tiled
    w2_d = nc.dram_tensor("w2", [32, 128, 1024], bf16, kind="ExternalInput")  # natural rows
    cos_d = nc.dram_tensor("cos2", [128, S], bf16, kind="ExternalInput")
    sin_d = nc.dram_tensor("sin2", [128, S], bf16, kind="ExternalInput")
    mod_d = nc.dram_tensor("mod", [6, D_MODEL], bf16, kind="ExternalInput")
    maskb_d = nc.dram_tensor("maskb", [128, 8], fp32, kind="ExternalInput")
    out_d = nc.dram_tensor("out", [SH, D_MODEL], fp32, kind="ExternalOutput")
    bias_d = {}
    if bq_nz:
        bias_d["bq"] = nc.dram_tensor("bq", [128, 8], fp32, kind="ExternalInput")
    if bk_nz:
        bias_d["bk"] = nc.dram_tensor("bk", [128, 8], fp32, kind="ExternalInput")
    if bv_nz:
        bias_d["bv"] = nc.dram_tensor("bv", [D_MODEL], fp32, kind="ExternalInput")
    if bo_nz:
        bias_d["bo"] = nc.dram_tensor("bo", [D_MODEL], fp32, kind="ExternalInput")
    if b1_nz:
        bias_d["b1"] = nc.dram_tensor("b1", [128, 32], fp32, kind="ExternalInput")
    if b2_nz:
        bias_d["b2"] = nc.dram_tensor("b2", [D_MODEL], fp32, kind="ExternalInput")

    with ExitStack() as ctx:
        tc = ctx.enter_context(tile.TileContext(nc))

        const = ctx.enter_context(tc.tile_pool(name="const", bufs=1))
        ident = const.tile([128, 128], bf16)
        make_identity(nc, ident[:])
        cos2 = const.tile([128, S], bf16)
        nc.gpsimd.dma_start(out=cos2[:], in_=cos_d[:])
        sin2 = const.tile([128, S], bf16)
        nc.gpsimd.dma_start(out=sin2[:], in_=sin_d[:])
        maskb = const.tile([128, 8], fp32)
        nc.gpsimd.dma_start(out=maskb[:], in_=maskb_d[:])
        eps_t = const.tile([128, 1], fp32)
        nc.vector.memset(eps_t[:], LN_EPS)
        ones_k = const.tile([128, 64], bf16)
        nc.vector.memset(ones_k[:], 1.0)
        # adaLN modulation vectors, broadcast across partitions
        mods = []
        for i in range(6):
            m = const.tile([128, D_MODEL], bf16, tag=f"mod{i}")
            nc.gpsimd.dma_start(out=m[:], in_=bass.AP(tensor=mod_d, offset=i * D_MODEL,
                                                      ap=[[0, 128], [1, D_MODEL]]))
            mods.append(m)
        m0b, b0b, a0b, m1b, b1mb, a1b = mods
        bias_sb = {}
        for nm in ("bq", "bk", "b1"):
            if nm in bias_d:
                t = const.tile(list(bias_d[nm].shape), fp32, tag=f"bias_{nm}")
                nc.sync.dma_start(out=t[:], in_=bias_d[nm][:])
                bias_sb[nm] = t
        for nm in ("bv", "bo", "b2"):
            if nm in bias_d:
                t = const.tile([128, D_MODEL], fp32, tag=f"bias_{nm}")
                nc.sync.dma_start(out=t[:], in_=bass.AP(tensor=bias_d[nm], offset=0,
                                                        ap=[[0, 128], [1, D_MODEL]]))
                bias_sb[nm] = t

        x_q = ctx.enter_context(tc.tile_pool(name="xq", bufs=1)).tile(
            [128, 4, D_MODEL], fp32)
        x1 = ctx.enter_context(tc.tile_pool(name="x1", bufs=1)).tile(
            [128, 4, D_MODEL], fp32)
        O_sb = ctx.enter_context(tc.tile_pool(name="attnO", bufs=1)).tile(
            [128, 8, SH], bf16)  # O^T concat [d, q]

        def layernorm_mod(x_t, pool, mbt, bbt, tagsfx):
            """LN over free axis + adaLN modulation; returns bf16 [128, D]."""
            stats = pool.tile([128, 2, 6], fp32, tag="stats" + tagsfx)
            nc.vector.bn_stats(out=stats[:, 0, :], in_=x_t[:, 0:512])
            nc.vector.bn_stats(out=stats[:, 1, :], in_=x_t[:, 512:1024])
            mv = pool.tile([128, 2], fp32, tag="mv" + tagsfx)
            nc.vector.bn_aggr(out=mv[:], in_=stats[:])
            std = pool.tile([128, 1], fp32, tag="std" + tagsfx)
            nc.scalar.activation(std[:], mv[:, 1:2], AF.Sqrt, bias=eps_t[:])
            rstd = pool.tile([128, 1], fp32, tag="rstd" + tagsfx)
            nc.vector.reciprocal(rstd[:], std[:])
            nrm = pool.tile([128, D_MODEL], bf16, tag="nrm" + tagsfx)
            nc.vector.tensor_scalar(out=nrm[:], in0=x_t, scalar1=mv[:, 0:1],
                                    scalar2=rstd[:], op0=OP.subtract, op1=OP.mult)
            t1 = pool.tile([128, D_MODEL], bf16, tag="t1" + tagsfx)
            nc.vector.tensor_mul(t1[:], nrm[:], mbt[:])
            nm_ = pool.tile([128, D_MODEL], bf16, tag="nm" + tagsfx)
            nc.vector.tensor_add(nm_[:], t1[:], bbt[:])
            return nm_

        def rope_apply(dst, src, n, pool):
            # dst, src: [128, n] bf16; rotate-half RoPE with sign-folded tables.
            # The rotate-half partition swap must go through DMA (DVE lanes
            # are partition-locked); spread the 4 slab copies over two queues.
            swp = pool.tile([128, n], bf16, tag="ropeswp")
            for eng, lo, sl in ((nc.gpsimd, 0, 32), (nc.scalar, 32, 0),
                                (nc.gpsimd, 64, 96), (nc.scalar, 96, 64)):
                eng.dma_start(out=swp[lo:lo + 32, :], in_=src[sl:sl + 32, :])
            tcos = pool.tile([128, n], bf16, tag="ropecos")
            nc.vector.tensor_mul(tcos[:], src, cos2[:, 0:n])
            tsin = pool.tile([128, n], bf16, tag="ropesin")
            nc.vector.tensor_mul(tsin[:], swp[:], sin2[:, 0:n])
            nc.vector.tensor_add(dst, tcos[:], tsin[:])

        with tc.tile_pool(name="kqv", bufs=1) as kqvp:
            Qt = kqvp.tile([128, 8, SH], bf16)       # Q~^T: [do, q]
            Kt = kqvp.tile([128, 8, S], bf16)        # K~^T: [do, k]
            Vn = kqvp.tile([128, 8, HEADS, HEAD_DIM], bf16)  # V natural

            with tc.tile_pool(name="n1t", bufs=1) as n1tp:
                n1T = n1tp.tile([128, 8, 1024], bf16)   # n1^T: [d, s]

                # ---------- Phase A: LN1 + modulation + transpose ----------
                with tc.tile_pool(name="ln1", bufs=3) as ln1p, \
                     tc.tile_pool(name="ln1ps", bufs=2, space="PSUM") as lnps:
                    for st in range(8):
                        if st < 4:
                            x_t = x_q[:, st, :]
                        else:
                            xkv = ln1p.tile([128, D_MODEL], fp32, tag="xkv")
                            x_t = xkv[:]
                        nc.sync.dma_start(out=x_t[:, 0:512],
                                          in_=x_d[st * 128:(st + 1) * 128, 0:512])
                        nc.scalar.dma_start(out=x_t[:, 512:1024],
                                            in_=x_d[st * 128:(st + 1) * 128, 512:1024])
                        n1m = layernorm_mod(x_t, ln1p, m0b, b0b, "1")
                        for dt in range(8):
                            tps = lnps.tile([128, 128], bf16, tag="tps")
                            nc.tensor.transpose(tps[:], n1m[:, dt * 128:(dt + 1) * 128],
                                                ident[:])
                            nc.scalar.copy(out=n1T[:, dt, st * 128:(st + 1) * 128],
                                           in_=tps[:])

                # ---------- Phase B: QKV projections (+RoPE) ----------
                with tc.tile_pool(name="wstream", bufs=5) as wsp, \
                     tc.tile_pool(name="wv", bufs=1) as wvp, \
                     tc.tile_pool(name="qkvtmp", bufs=3) as qtp, \
                     tc.tile_pool(name="qkvps", bufs=3, space="PSUM") as qps:
                    # Q^T[do, q], q = first 512 rolled rows
                    for m in range(8):
                        wqt = wsp.tile([128, 1024], bf16, tag="wt")
                        nc.sync.dma_start(out=wqt[:], in_=wq_d[m])
                        ps = qps.tile([128, SH], fp32, tag="projps")
                        for k in range(8):
                            nc.tensor.matmul(ps[:], wqt[:, k * 128:(k + 1) * 128],
                                             n1T[:, k, 0:SH],
                                             start=(k == 0), stop=(k == 7))
                        qtmp = qtp.tile([128, SH], bf16, tag="qtmp")
                        if bq_nz:
                            nc.scalar.activation(qtmp[:], ps[:], AF.Copy,
                                                 bias=bias_sb["bq"][:, m:m + 1])
                        else:
                            nc.scalar.copy(out=qtmp[:], in_=ps[:])
                        rope_apply(Qt[:, m, :], qtmp[:], SH, qtp)

                    # K^T[do, k] over all 1024 rolled rows
                    for m in range(8):
                        wkt = wsp.tile([128, 1024], bf16, tag="wt")
                        nc.sync.dma_start(out=wkt[:], in_=wk_d[m])
                        ktmp = qtp.tile([128, S], bf16, tag="ktmp")
                        for nh in range(2):
                            ps = qps.tile([128, SH], fp32, tag="projps")
                            for k in range(8):
                                nc.tensor.matmul(ps[:], wkt[:, k * 128:(k + 1) * 128],
                                                 n1T[:, k, nh * SH:(nh + 1) * SH],
                                                 start=(k == 0), stop=(k == 7))
                            if bk_nz:
                                nc.scalar.activation(ktmp[:, nh * SH:(nh + 1) * SH],
                                                     ps[:], AF.Copy,
                                                     bias=bias_sb["bk"][:, m:m + 1])
                            else:
                                nc.scalar.copy(out=ktmp[:, nh * SH:(nh + 1) * SH],
                                               in_=ps[:])
                        rope_apply(Kt[:, m, :], ktmp[:], S, qtp)

                    # V natural [s, dv] (+ ones column per head)
                    wv_sb = wvp.tile([128, 8, 1024], bf16)
                    for k in range(8):
                        nc.sync.dma_start(out=wv_sb[:, k, :], in_=wv_d[k])
                    for st in range(8):
                        for nh in range(2):
                            ps = qps.tile([128, SH], fp32, tag="projps")
                            for k in range(8):
                                nc.tensor.matmul(ps[:],
                                                 n1T[:, k, st * 128:(st + 1) * 128],
                                                 wv_sb[:, k, nh * SH:(nh + 1) * SH],
                                                 start=(k == 0), stop=(k == 7))
                            src = ps[:]
                            if bv_nz:
                                vtmp = qtp.tile([128, SH], fp32, tag="vtmp")
                                nc.vector.tensor_add(
                                    vtmp[:], ps[:],
                                    bias_sb["bv"][:, nh * SH:(nh + 1) * SH])
                                src = vtmp[:]
                            nc.vector.tensor_copy(
                                out=Vn[:, st, nh * 8:(nh + 1) * 8, :],
                                in_=src.rearrange("p (h d) -> p h d", d=HEAD_DIM))

            # ---------- Phase C: attention ----------
            # Heads run in even/odd pairs. Per pair: scores for both heads
            # (PE row groups 0:64 / 64:128 work concurrently), exp on 2-bank
            # PSUM tiles, then attnV + denominator MMs packed into single
            # [128,512] PSUM banks (even head rows 0:64, odd head rows 64:128
            # via tile_position) so normalization is ONE wide reciprocal and
            # ONE wide multiply writing O_sb directly.
            with tc.tile_pool(name="pt", bufs=2) as ptp, \
                 tc.tile_pool(name="dn", bufs=2) as dnp, \
                 tc.tile_pool(name="spsp", bufs=2, space="PSUM") as spsp, \
                 tc.tile_pool(name="ovdn", bufs=4, space="PSUM") as ovdnp:
                for pr in range(HEADS // 2):
                    mt = pr
                    PTs = [ptp.tile([128, 8, SH], bf16, tag=f"PT{par}",
                                    name=f"PT_{pr}_{par}") for par in range(2)]
                    for kb2 in range(4):
                        sps2 = [spsp.tile([128, 2, SH], fp32, tag="sps",
                                          name=f"sps_{pr}_{kb2}_{par}")
                                for par in range(2)]
                        for sub in range(2):
                            kb = 2 * kb2 + sub
                            for par in range(2):
                                po = par * 64
                                nc.tensor.matmul(
                                    sps2[par][:, sub, :],
                                    Kt[po:po + 64, mt, kb * 128:(kb + 1) * 128],
                                    Qt[po:po + 64, mt, :])
                        for par in range(2):
                            if mtriv:
                                nc.scalar.activation(
                                    PTs[par][:, 2 * kb2:2 * kb2 + 2, :],
                                    sps2[par][:], AF.Exp,
                                    scale=float(1.0 / np.sqrt(HEAD_DIM)))
                            else:
                                for sub in range(2):
                                    kb = 2 * kb2 + sub
                                    nc.scalar.activation(
                                        PTs[par][:, kb, :], sps2[par][:, sub, :],
                                        AF.Exp, bias=maskb[:, kb:kb + 1],
                                        scale=float(1.0 / np.sqrt(HEAD_DIM)))
                    ov2 = ovdnp.tile([128, SH], fp32, tag="ovdn",
                                     name=f"ov_{pr}")
                    dn2 = ovdnp.tile([128, SH], fp32, tag="ovdn",
                                     name=f"dn_{pr}")
                    for kb in range(8):
                        st = (kb == 0)
                        sp = (kb == 7)
                        nc.tensor.matmul(ov2[0:64, :], Vn[:, kb, 2 * pr, :],
                                         PTs[0][:, kb, :], start=st, stop=sp,
                                         skip_group_check=True)
                        nc.tensor.matmul(ov2[64:128, :], Vn[:, kb, 2 * pr + 1, :],
                                         PTs[1][:, kb, :], start=st, stop=sp,
                                         tile_position=(0, 64),
                                         skip_group_check=True)
                        nc.tensor.matmul(dn2[0:64, :], ones_k[:],
                                         PTs[0][:, kb, :], start=st, stop=sp,
                                         skip_group_check=True)
                        nc.tensor.matmul(dn2[64:128, :], ones_k[:],
                                         PTs[1][:, kb, :], start=st, stop=sp,
                                         tile_position=(0, 64),
                                         skip_group_check=True)
                    rrec = dnp.tile([128, SH], fp32, tag="rrec",
                                    name=f"rrec_{pr}")
                    nc.vector.reciprocal(rrec[:], dn2[:])
                    nc.vector.tensor_mul(O_sb[:, mt, :], ov2[:], rrec[:])

        # ---------- Phase D: output projection + residual + LN2,
        # interleaved per qb-block so LN2's DVE work and transposes overlap
        # the next qb's out-projection matmuls ----------
        with tc.tile_pool(name="ffn", bufs=1) as ffnp:
            n2T = ffnp.tile([128, 8, SH], bf16)
            hT = ffnp.tile([128, 32, SH], bf16)

            with tc.tile_pool(name="wo", bufs=1) as wop, \
                 tc.tile_pool(name="optmp", bufs=3) as opt, \
                 tc.tile_pool(name="ln2", bufs=2) as ln2p, \
                 tc.tile_pool(name="ops", bufs=3, space="PSUM") as opsp, \
                 tc.tile_pool(name="ln2ps", bufs=2, space="PSUM") as ln2ps:
                wo_sb = wop.tile([128, 8, 1024], bf16)
                for k in range(8):
                    nc.sync.dma_start(out=wo_sb[:, k, :], in_=wo_d[k])
                for qb in range(4):
                    for nh in range(2):
                        ps = opsp.tile([128, SH], fp32, tag="ops")
                        for k in range(8):
                            nc.tensor.matmul(ps[:],
                                             O_sb[:, k, qb * 128:(qb + 1) * 128],
                                             wo_sb[:, k, nh * SH:(nh + 1) * SH],
                                             start=(k == 0), stop=(k == 7))
                        sl = slice(nh * SH, (nh + 1) * SH)
                        t = opt.tile([128, SH], fp32, tag="opt")
                        if bo_nz:
                            t2 = opt.tile([128, SH], fp32, tag="opt2")
                            nc.vector.tensor_add(t2[:], ps[:], bias_sb["bo"][:, sl])
                            nc.vector.tensor_mul(t[:], t2[:], a0b[:, sl])
                        else:
                            nc.vector.tensor_mul(t[:], ps[:], a0b[:, sl])
                        nc.vector.tensor_add(x1[:, qb, sl], t[:], x_q[:, qb, sl])
                    n2m = layernorm_mod(x1[:, qb, :], ln2p, m1b, b1mb, "2")
                    for dt in range(8):
                        tps = ln2ps.tile([128, 128], bf16, tag="tps2")
                        nc.tensor.transpose(tps[:], n2m[:, dt * 128:(dt + 1) * 128],
                                            ident[:])
                        nc.scalar.copy(out=n2T[:, dt, qb * 128:(qb + 1) * 128],
                                       in_=tps[:])

            # FFN1: hT[dff, q] = gelu(w1^T @ n2^T)
            with tc.tile_pool(name="w1s", bufs=6) as w1p, \
                 tc.tile_pool(name="f1ps", bufs=2, space="PSUM") as f1ps:
                for j in range(32):
                    w1t = w1p.tile([128, 1024], bf16, tag="w1t")
                    nc.sync.dma_start(out=w1t[:], in_=w1_d[j])
                    ps = f1ps.tile([128, SH], fp32, tag="f1")
                    for k in range(8):
                        nc.tensor.matmul(ps[:], w1t[:, k * 128:(k + 1) * 128],
                                         n2T[:, k, :], start=(k == 0), stop=(k == 7))
                    if b1_nz:
                        nc.scalar.activation(hT[:, j, :], ps[:], AF.Gelu,
                                             bias=bias_sb["b1"][:, j:j + 1])
                    else:
                        nc.scalar.activation(hT[:, j, :], ps[:], AF.Gelu)

            # FFN2: y[q, do] accumulated over j, in staggered qb-groups so
            # earlier groups' epilogues + output DMAs overlap later groups'
            # matmuls (w2 is streamed per group; extra HBM reads, hidden).
            with tc.tile_pool(name="w2s", bufs=6) as w2p, \
                 tc.tile_pool(name="f2ps", bufs=1, space="PSUM") as f2ps, \
                 tc.tile_pool(name="otmp", bufs=2) as otp:
                for grp, qbs in enumerate(((0, 1), (2,), (3,))):
                    psl = {(qb, nh): f2ps.tile([128, SH], fp32,
                                               tag=f"f2_{qb}_{nh}",
                                               name=f"f2_{qb}_{nh}")
                           for qb in qbs for nh in range(2)}
                    for j in range(32):
                        w2t = w2p.tile([128, 1024], bf16, tag="w2t")
                        nc.sync.dma_start(out=w2t[:], in_=w2_d[j])
                        for qb in qbs:
                            for nh in range(2):
                                nc.tensor.matmul(psl[qb, nh][:],
                                                 hT[:, j, qb * 128:(qb + 1) * 128],
                                                 w2t[:, nh * SH:(nh + 1) * SH],
                                                 start=(j == 0), stop=(j == 31))
                    for qb in qbs:
                        for nh in range(2):
                            sl = slice(nh * SH, (nh + 1) * SH)
                            ps = psl[qb, nh]
                            t = otp.tile([128, SH], fp32, tag="ot")
                            if b2_nz:
                                t2 = otp.tile([128, SH], fp32, tag="ot2")
                                nc.vector.tensor_add(t2[:], ps[:],
                                                     bias_sb["b2"][:, sl])
                                nc.vector.tensor_mul(t[:], t2[:], a1b[:, sl])
                            else:
                                nc.vector.tensor_mul(t[:], ps[:], a1b[:, sl])
                            yo = otp.tile([128, SH], fp32, tag="yo")
                            nc.vector.tensor_add(yo[:], t[:], x1[:, qb, sl])
                            eng = (nc.gpsimd, nc.scalar)[(qb * 2 + nh) % 2]
                            eng.dma_start(out=out_d[qb * 128:(qb + 1) * 128, sl],
                                          in_=yo[:])

    nc.compile()
    return nc


def _lhsT_tile(w, nblocks_in, nblocks_out):
    # w: [in, out] -> [nblocks_out, 128, nblocks_in*128] with
    # result[m][p, k*128+c] = w[k*128+p, m*128+c]
    kin = w.shape[0] // nblocks_in
    return np.ascontiguousarray(
        w.reshape(nblocks_in, kin, nblocks_out, w.shape[1] // nblocks_out)
        .transpose(2, 1, 0, 3)
        .reshape(nblocks_out, kin, -1))


def kernel(src_reps, src_mask, compact_style,
           ada0_w, ada0_b, ada1_w, ada1_b,
           wq, bq, wk, bk, wv, bv, wo, bo,
           w1, b1, w2, b2):
    trace = bool(os.environ.get("KERNEL_TRACE"))
    if trace:
        _install_ntff_shim()
    from concourse.bass_utils import run_bass_kernel_spmd

    src_reps = np.asarray(src_reps, np.float32)
    src_mask = np.asarray(src_mask)
    compact_style = np.asarray(compact_style, np.float32)

    # ---- host prep: adaLN styles ----
    def styles(ada_w, ada_b):
        cs = compact_style
        silu = cs * (1.0 / (1.0 + np.exp(-cs)))
        st = silu @ np.asarray(ada_w, np.float32) + np.asarray(ada_b, np.float32)
        g, be, al = st[:, :D_MODEL], st[:, D_MODEL:2 * D_MODEL], st[:, 2 * D_MODEL:]
        return (1.0 + np.tanh(g) * GAMMA_SCALE), be, al

    m0, be0, al0 = styles(ada0_w, ada0_b)
    m1, be1, al1 = styles(ada1_w, ada1_b)

    # ---- host prep: weights (cast + tile) ----
    wq_l = _lhsT_tile(np.asarray(wq), 8, 8).astype(_BF16)
    wk_l = _lhsT_tile(np.asarray(wk), 8, 8).astype(_BF16)
    wv_n = np.ascontiguousarray(np.asarray(wv).reshape(8, 128, 1024)).astype(_BF16)
    wo_n = np.ascontiguousarray(np.asarray(wo).reshape(8, 128, 1024)).astype(_BF16)
    w1_l = _lhsT_tile(np.asarray(w1), 8, 32).astype(_BF16)
    w2_n = np.ascontiguousarray(np.asarray(w2).reshape(32, 128, 1024)).astype(_BF16)

    flags = (bool(np.all(src_mask)),) + tuple(
        bool(np.any(np.asarray(b) != 0)) for b in (bq, bk, bv, bo, b1, b2))
    if flags not in _graph_cache:
        _graph_cache[flags] = _build_graph(flags)
    nc = _graph_cache[flags]

    # ---- host prep: RoPE tables (per roll offset) ----
    inv_freq = 1.0 / (ROPE_BASE **
                      (np.arange(0, HEAD_DIM, 2, dtype=np.float32) / HEAD_DIM))
    d_in_head = np.arange(64)
    fidx = np.where(d_in_head < 32, d_in_head, d_in_head - 32)
    sign = np.where(d_in_head < 32, -1.0, 1.0).astype(np.float32)

    def rope_tables(roll):
        pos = np.roll(np.arange(S, dtype=np.float32), -roll)
        ang = pos[None, :] * inv_freq[fidx][:, None]  # [64, S]
        c = np.cos(ang).astype(np.float32)
        s_ = (np.sin(ang) * sign[:, None]).astype(np.float32)
        return (np.ascontiguousarray(np.concatenate([c, c], 0)).astype(_BF16),
                np.ascontiguousarray(np.concatenate([s_, s_], 0)).astype(_BF16))

    tables = [rope_tables(0), rope_tables(SH)]

    in_maps = []
    for c in range(N_CORES):
        b, h = c // 2, c % 2
        x_c = np.ascontiguousarray(np.roll(src_reps[b], -h * SH, axis=0))
        mb = np.where(np.roll(src_mask[b], -h * SH), 0.0, -60.0).astype(np.float32)
        mod = np.stack([m0[b], be0[b], al0[b], m1[b], be1[b], al1[b]])
        im = {
            "x": x_c, "wq": wq_l, "wk": wk_l, "wv": wv_n, "wo": wo_n,
            "w1": w1_l, "w2": w2_n,
            "cos2": tables[h][0], "sin2": tables[h][1],
            "mod": np.ascontiguousarray(mod.astype(_BF16)),
            "maskb": np.ascontiguousarray(mb.reshape(8, 128).T),
        }
        if flags[1]:
            im["bq"] = np.ascontiguousarray(np.asarray(bq, np.float32).reshape(8, 128).T)
        if flags[2]:
            im["bk"] = np.ascontiguousarray(np.asarray(bk, np.float32).reshape(8, 128).T)
        if flags[3]:
            im["bv"] = np.asarray(bv, np.float32)
        if flags[4]:
            im["bo"] = np.asarray(bo, np.float32)
        if flags[5]:
            im["b1"] = np.ascontiguousarray(np.asarray(b1, np.float32).reshape(32, 128).T)
        if flags[6]:
            im["b2"] = np.asarray(b2, np.float32)
        in_maps.append(im)

    res = run_bass_kernel_spmd(nc, in_maps, core_ids=list(range(N_CORES)),
                               trace=trace)
    kernel.last_result = res

    out = np.empty((B, S, D_MODEL), np.float32)
    for c in range(N_CORES):
        b, h = c // 2, c % 2
        out[b, h * SH:(h + 1) * SH, :] = res.results[c]["out"]
    return out
